# revision 6
# baseline (speedup 1.0000x reference)
"""Trainium2 Bass kernel for nn_KagomeSignNetwork.

Strategy
--------
The reference network is, per sample b:
    h0[b]   = spins(x[b])                       in {-1,+1}^36
    h_{l+1} = relu(Conv_l(h_l))                 4 graph-conv layers, C=32
    out     = readout(h4)                       sublattice-mean + linear -> 2

Each graph conv  out[b,o,n] = sum_{c,k} h[b,c,nbr[n,k]] * W[subl[n],o,c,k] + b
is LINEAR in the flattened feature vector f = (c*36 + site).  We therefore
fold the neighbor gather + per-sublattice weight selection into one dense
matrix M_l [1152, 1152] on the host (and the bit-unpack / readout likewise),
turning the whole network into a dense MLP:

    h1 = relu(M0 @ s + B0)        M0: [1152, 36]
    h2 = relu(M1 @ h1 + B1)       M1..M3: [1152, 1152]
    h3 = relu(M2 @ h2 + B2)
    h4 = relu(M3 @ h3 + B3)
    out = R @ h4 + bt             R: [2, 1152]

This maps perfectly onto the 128x128 PE array (no gathers on device).
Matmuls run in fp16 (weights+activations) with fp32 PSUM accumulation:
measured end-to-end rel-l2 error vs the fp32 reference is ~2.6e-4.

Sharding: pure data parallelism — batch 16384 is split 8 x 2048; weights
replicated.  Activations live entirely in SBUF as [features, batch] tiles
(feature chunk on the 128-partition dim), so HBM traffic is just weights
(~8 MB/core) + spins in + logits out.
"""

import numpy as np

import concourse.bass as bass
import concourse.mybir as mybir
from concourse import bacc
from concourse.bass_utils import run_bass_kernel_spmd
from concourse.tile import TileContext

B, N, C = 16384, 36, 32
F = C * N            # 1152 features per layer
NCORES = 8
BC = B // NCORES     # 2048 samples per core
KC = F // 128        # 9 feature chunks of 128
NB = BC // 512       # 4 batch chunks of 512
FP16 = mybir.dt.float16
FP32 = mybir.dt.float32

_COMPILED = {}


def _build_program():
    """Build + compile the SPMD Bass program (cached per process)."""
    if "nc" in _COMPILED:
        return _COMPILED["nc"]

    nc = bacc.Bacc("TRN2", target_bir_lowering=False, debug=False,
                   num_devices=NCORES)

    sT_d = nc.declare_dram_parameter("sT", [N, BC], FP16, isOutput=False)
    w0_d = nc.declare_dram_parameter("w0", [N, F], FP16, isOutput=False)
    w_d = [nc.declare_dram_parameter(f"w{l}", [128, KC * F], FP16,
                                     isOutput=False) for l in (1, 2, 3)]
    rt_d = nc.declare_dram_parameter("rt", [128, KC * 2], FP16, isOutput=False)
    bias_d = nc.declare_dram_parameter("bias", [128, 4 * KC], FP32,
                                       isOutput=False)
    bt_d = nc.declare_dram_parameter("bt", [2, 1], FP32, isOutput=False)
    out_d = nc.declare_dram_parameter("outT", [2, BC], FP32, isOutput=True)

    with TileContext(nc) as tc:
        with (
            tc.tile_pool(name="const", bufs=1) as cpool,
            tc.tile_pool(name="hbuf", bufs=2) as hpool,
            tc.tile_pool(name="psum", bufs=6, space="PSUM") as pspool,
            tc.tile_pool(name="psr", bufs=2, space="PSUM") as psrpool,
        ):
            # ---- resident constants / weights ----
            sT = cpool.tile([N, BC], FP16, tag="sT")
            nc.sync.dma_start(out=sT[:], in_=sT_d[:])
            w0 = cpool.tile([N, F], FP16, tag="w0")
            nc.sync.dma_start(out=w0[:], in_=w0_d[:])
            bias = cpool.tile([128, 4 * KC], FP32, tag="bias")
            nc.sync.dma_start(out=bias[:], in_=bias_d[:])
            bt = cpool.tile([2, 1], FP32, tag="bt")
            nc.sync.dma_start(out=bt[:], in_=bt_d[:])
            rt = cpool.tile([128, KC * 2], FP16, tag="rt")
            nc.sync.dma_start(out=rt[:], in_=rt_d[:])
            ws = []
            for l in range(3):
                # SBUF col kc*F + mo*128 + j  <-  MT[kc*128+p, mo*128+j]
                # (host pre-arranges w_d as [p, kc, c] so this is contiguous)
                w = cpool.tile([128, KC * F], FP16, tag=f"wl{l}")
                nc.sync.dma_start(out=w[:], in_=w_d[l][:])
                ws.append(w)

            relu = mybir.ActivationFunctionType.Relu
            ident = mybir.ActivationFunctionType.Identity

            # ---- layer 0: K=36, s -> h1 ----
            h_prev = hpool.tile([128, KC * BC], FP16, tag="h")
            for mo in range(KC):
                for nb in range(NB):
                    ps = pspool.tile([128, 512], FP32, tag="ps")
                    nc.tensor.matmul(
                        ps[:],
                        lhsT=w0[:, mo * 128:(mo + 1) * 128],
                        rhs=sT[:, nb * 512:(nb + 1) * 512],
                        start=True, stop=True,
                    )
                    nc.scalar.activation(
                        h_prev[:, mo * BC + nb * 512: mo * BC + (nb + 1) * 512],
                        ps[:], relu, bias=bias[:, mo:mo + 1],
                    )

            # ---- layers 1..3: dense 1152 -> 1152 ----
            for l in range(3):
                w = ws[l]
                h_next = hpool.tile([128, KC * BC], FP16, tag="h")
                for mo in range(KC):
                    for nb in range(NB):
                        ps = pspool.tile([128, 512], FP32, tag="ps")
                        for kc in range(KC):
                            nc.tensor.matmul(
                                ps[:],
                                lhsT=w[:, kc * F + mo * 128:
                                       kc * F + (mo + 1) * 128],
                                rhs=h_prev[:, kc * BC + nb * 512:
                                           kc * BC + (nb + 1) * 512],
                                start=(kc == 0), stop=(kc == KC - 1),
                            )
                        nc.scalar.activation(
                            h_next[:, mo * BC + nb * 512:
                                   mo * BC + (nb + 1) * 512],
                            ps[:], relu,
                            bias=bias[:, (l + 1) * KC + mo:(l + 1) * KC + mo + 1],
                        )
                h_prev = h_next

            # ---- readout: [2, 1152] @ h4 + bt ----
            outT = cpool.tile([2, BC], FP32, tag="outT")
            for nb in range(NB):
                ps = psrpool.tile([2, 512], FP32, tag="psr")
                for kc in range(KC):
                    nc.tensor.matmul(
                        ps[:],
                        lhsT=rt[:, kc * 2:(kc + 1) * 2],
                        rhs=h_prev[:, kc * BC + nb * 512:
                                   kc * BC + (nb + 1) * 512],
                        start=(kc == 0), stop=(kc == KC - 1),
                    )
                nc.scalar.activation(
                    outT[:, nb * 512:(nb + 1) * 512], ps[:], ident,
                    bias=bt[:],
                )
            nc.sync.dma_start(out=out_d[:], in_=outT[:])

    nc.compile()
    _COMPILED["nc"] = nc
    return nc


def _build_dense(W, subl, nbr):
    """M[(o*N+n), (c*N+m)] = sum_k W[subl[n],o,c,k] * [nbr[n,k]==m]."""
    Cout, Cin = W.shape[1], W.shape[2]
    Ws = W[subl]                                     # [N, Cout, Cin, 15]
    M = np.zeros((Cout, N, Cin, N), dtype=np.float32)
    n_idx = np.repeat(np.arange(N), nbr.shape[1])
    m_idx = nbr.reshape(-1).astype(np.int64)
    vals = Ws.transpose(0, 3, 1, 2).reshape(N * nbr.shape[1], Cout, Cin)
    Mp = M.transpose(1, 3, 0, 2)                     # [N_out, N_in, Cout, Cin]
    np.add.at(Mp, (n_idx, m_idx), vals.astype(np.float32))
    return M.reshape(Cout * N, Cin * N)


def kernel(x, W0, b0, W1, b1, W2, b2, W3, b3, Wt, bt, nbr, subl, red):
    x = np.asarray(x)
    nbr = np.asarray(nbr).astype(np.int32)
    subl = np.asarray(subl).astype(np.int32)
    red = np.asarray(red).astype(np.int32)
    W = [np.asarray(w, dtype=np.float32) for w in (W0, W1, W2, W3)]
    bvec = [np.asarray(b, dtype=np.float32) for b in (b0, b1, b2, b3)]
    Wt = np.asarray(Wt, dtype=np.float32)
    bt = np.asarray(bt, dtype=np.float32)

    # ---- host: fold gather/sublattice/readout into dense matrices ----
    M0 = _build_dense(W[0], subl, nbr)               # [1152, 36]
    Ms = [_build_dense(W[l], subl, nbr) for l in (1, 2, 3)]
    Bs = [b[subl].T.reshape(-1).astype(np.float32) for b in bvec]  # [1152]
    Rm = np.zeros((2, C, N), dtype=np.float32)
    for t in range(3):
        for i in range(red.shape[1]):
            Rm[:, :, red[t, i]] += Wt[:, np.arange(C) * 3 + t] / red.shape[1]
    Rm = Rm.reshape(2, F)

    # spins, transposed to [36, B] fp16
    bits = (x[:, None] >> np.arange(N, dtype=x.dtype)) & 1
    sT_full = np.ascontiguousarray(
        (2 * bits - 1).astype(np.float16).T)         # [36, 16384]

    w0_h = np.ascontiguousarray(M0.T.astype(np.float16))        # [36, 1152]
    # [128p, kc*F + c] with w_h[p, kc*F + c] = MT[kc*128+p, c]
    w_h = [np.ascontiguousarray(
        M.T.astype(np.float16).reshape(KC, 128, F)
        .transpose(1, 0, 2).reshape(128, KC * F))
        for M in Ms]
    rt_h = np.zeros((128, KC * 2), dtype=np.float16)            # RT tiled
    RT = Rm.T.astype(np.float16)                                # [1152, 2]
    for kc in range(KC):
        rt_h[:, kc * 2:(kc + 1) * 2] = RT[kc * 128:(kc + 1) * 128, :]
    bias_h = np.zeros((128, 4 * KC), dtype=np.float32)
    for l in range(4):
        for mo in range(KC):
            bias_h[:, l * KC + mo] = Bs[l][mo * 128:(mo + 1) * 128]
    bt_h = bt.reshape(2, 1).astype(np.float32)

    nc = _build_program()
    shared = {"w0": w0_h, "w1": w_h[0], "w2": w_h[1], "w3": w_h[2],
              "rt": rt_h, "bias": bias_h, "bt": bt_h}
    in_maps = []
    for c in range(NCORES):
        m = dict(shared)
        m["sT"] = np.ascontiguousarray(sT_full[:, c * BC:(c + 1) * BC])
        in_maps.append(m)

    global _LAST_IN_MAPS
    _LAST_IN_MAPS = in_maps
    res = run_bass_kernel_spmd(nc, in_maps, list(range(NCORES)))

    out = np.empty((B, 2), dtype=np.float32)
    for c in range(NCORES):
        out[c * BC:(c + 1) * BC] = res.results[c]["outT"].T
    return out


# revision 9
# speedup vs baseline: 1.0054x; 1.0054x over previous
"""Trainium2 Bass kernel for nn_KagomeSignNetwork.

Strategy
--------
The reference network is, per sample b:
    h0[b]   = spins(x[b])                       in {-1,+1}^36
    h_{l+1} = relu(Conv_l(h_l))                 4 graph-conv layers, C=32
    out     = readout(h4)                       sublattice-mean + linear -> 2

Each graph conv  out[b,o,n] = sum_{c,k} h[b,c,nbr[n,k]] * W[subl[n],o,c,k] + b
is LINEAR in the flattened feature vector f = (c*36 + site).  We therefore
fold the neighbor gather + per-sublattice weight selection into one dense
matrix M_l [1152, 1152] on the host (and the bit-unpack / readout likewise),
turning the whole network into a dense MLP:

    h1 = relu(M0 @ s + B0)        M0: [1152, 36]
    h2 = relu(M1 @ h1 + B1)       M1..M3: [1152, 1152]
    h3 = relu(M2 @ h2 + B2)
    h4 = relu(M3 @ h3 + B3)
    out = R @ h4 + bt             R: [2, 1152]

This maps perfectly onto the 128x128 PE array (no gathers on device).
Matmuls run in fp16 (weights+activations) with fp32 PSUM accumulation:
measured end-to-end rel-l2 error vs the fp32 reference is ~2.6e-4.

Sharding: pure data parallelism — batch 16384 is split 8 x 2048; weights
replicated.  Activations live entirely in SBUF as [features, batch] tiles
(feature chunk on the 128-partition dim), so HBM traffic is just weights
(~8 MB/core) + spins in + logits out.
"""

import numpy as np

import concourse.bass as bass
import concourse.mybir as mybir
from concourse import bacc
from concourse.bass_utils import run_bass_kernel_spmd
from concourse.tile import TileContext

B, N, C = 16384, 36, 32
F = C * N            # 1152 features per layer
NCORES = 8
BC = B // NCORES     # 2048 samples per core
KC = F // 128        # 9 feature chunks of 128
NB = BC // 512       # 4 batch chunks of 512
import os
if os.environ.get("KERNEL_MM_DTYPE") == "bf16":
    import ml_dtypes
    FP16 = mybir.dt.bfloat16
    NP16 = np.dtype(ml_dtypes.bfloat16)
else:
    FP16 = mybir.dt.float16
    NP16 = np.dtype("float16")
FP32 = mybir.dt.float32

_COMPILED = {}


def _build_program():
    """Build + compile the SPMD Bass program (cached per process)."""
    if "nc" in _COMPILED:
        return _COMPILED["nc"]

    nc = bacc.Bacc("TRN2", target_bir_lowering=False, debug=False,
                   num_devices=NCORES)

    sT_d = nc.declare_dram_parameter("sT", [N, BC], FP16, isOutput=False)
    w0_d = nc.declare_dram_parameter("w0", [N, F], FP16, isOutput=False)
    w_d = [nc.declare_dram_parameter(f"w{l}", [128, KC * F], FP16,
                                     isOutput=False) for l in (1, 2, 3)]
    rt_d = nc.declare_dram_parameter("rt", [128, KC * 2], FP16, isOutput=False)
    bias_d = nc.declare_dram_parameter("bias", [128, 4 * KC], FP32,
                                       isOutput=False)
    bt_d = nc.declare_dram_parameter("bt", [2, 1], FP32, isOutput=False)
    out_d = nc.declare_dram_parameter("outT", [2, BC], FP32, isOutput=True)

    with TileContext(nc) as tc:
        with (
            tc.tile_pool(name="const", bufs=1) as cpool,
            tc.tile_pool(name="hbuf", bufs=2) as hpool,
            tc.tile_pool(name="psum", bufs=6, space="PSUM") as pspool,
            tc.tile_pool(name="psr", bufs=2, space="PSUM") as psrpool,
        ):
            # ---- resident constants / weights ----
            sT = cpool.tile([N, BC], FP16, tag="sT")
            nc.sync.dma_start(out=sT[:], in_=sT_d[:])
            w0 = cpool.tile([N, F], FP16, tag="w0")
            nc.sync.dma_start(out=w0[:], in_=w0_d[:])
            bias = cpool.tile([128, 4 * KC], FP32, tag="bias")
            nc.sync.dma_start(out=bias[:], in_=bias_d[:])
            bt = cpool.tile([2, 1], FP32, tag="bt")
            nc.sync.dma_start(out=bt[:], in_=bt_d[:])
            rt = cpool.tile([128, KC * 2], FP16, tag="rt")
            nc.sync.dma_start(out=rt[:], in_=rt_d[:])
            ws = []
            for l in range(3):
                # SBUF col kc*F + mo*128 + j  <-  MT[kc*128+p, mo*128+j]
                # (host pre-arranges w_d as [p, kc, c] so this is contiguous)
                w = cpool.tile([128, KC * F], FP16, tag=f"wl{l}")
                nc.sync.dma_start(out=w[:], in_=w_d[l][:])
                ws.append(w)

            relu = mybir.ActivationFunctionType.Relu
            ident = mybir.ActivationFunctionType.Identity

            # ---- layer 0: K=36, s -> h1 ----
            h_prev = hpool.tile([128, KC * BC], FP16, tag="h")
            for mo in range(KC):
                for nb in range(NB):
                    ps = pspool.tile([128, 512], FP32, tag="ps")
                    nc.tensor.matmul(
                        ps[:],
                        lhsT=w0[:, mo * 128:(mo + 1) * 128],
                        rhs=sT[:, nb * 512:(nb + 1) * 512],
                        start=True, stop=True,
                    )
                    nc.scalar.activation(
                        h_prev[:, mo * BC + nb * 512: mo * BC + (nb + 1) * 512],
                        ps[:], relu, bias=bias[:, mo:mo + 1],
                    )

            # ---- layers 1..3: dense 1152 -> 1152 ----
            for l in range(3):
                w = ws[l]
                h_next = hpool.tile([128, KC * BC], FP16, tag="h")
                for mo in range(KC):
                    for nb in range(NB):
                        ps = pspool.tile([128, 512], FP32, tag="ps")
                        for kc in range(KC):
                            nc.tensor.matmul(
                                ps[:],
                                lhsT=w[:, kc * F + mo * 128:
                                       kc * F + (mo + 1) * 128],
                                rhs=h_prev[:, kc * BC + nb * 512:
                                           kc * BC + (nb + 1) * 512],
                                start=(kc == 0), stop=(kc == KC - 1),
                            )
                        nc.scalar.activation(
                            h_next[:, mo * BC + nb * 512:
                                   mo * BC + (nb + 1) * 512],
                            ps[:], relu,
                            bias=bias[:, (l + 1) * KC + mo:(l + 1) * KC + mo + 1],
                        )
                h_prev = h_next

            # ---- readout: [2, 1152] @ h4 + bt ----
            outT = cpool.tile([2, BC], FP32, tag="outT")
            for nb in range(NB):
                ps = psrpool.tile([2, 512], FP32, tag="psr")
                for kc in range(KC):
                    nc.tensor.matmul(
                        ps[:],
                        lhsT=rt[:, kc * 2:(kc + 1) * 2],
                        rhs=h_prev[:, kc * BC + nb * 512:
                                   kc * BC + (nb + 1) * 512],
                        start=(kc == 0), stop=(kc == KC - 1),
                    )
                nc.scalar.activation(
                    outT[:, nb * 512:(nb + 1) * 512], ps[:], ident,
                    bias=bt[:],
                )
            nc.sync.dma_start(out=out_d[:], in_=outT[:])

    nc.compile()
    _COMPILED["nc"] = nc
    return nc


def _build_dense(W, subl, nbr):
    """M[(o*N+n), (c*N+m)] = sum_k W[subl[n],o,c,k] * [nbr[n,k]==m]."""
    Cout, Cin = W.shape[1], W.shape[2]
    Ws = W[subl]                                     # [N, Cout, Cin, 15]
    M = np.zeros((Cout, N, Cin, N), dtype=np.float32)
    n_idx = np.repeat(np.arange(N), nbr.shape[1])
    m_idx = nbr.reshape(-1).astype(np.int64)
    vals = Ws.transpose(0, 3, 1, 2).reshape(N * nbr.shape[1], Cout, Cin)
    Mp = M.transpose(1, 3, 0, 2)                     # [N_out, N_in, Cout, Cin]
    np.add.at(Mp, (n_idx, m_idx), vals.astype(np.float32))
    return M.reshape(Cout * N, Cin * N)


def kernel(x, W0, b0, W1, b1, W2, b2, W3, b3, Wt, bt, nbr, subl, red):
    x = np.asarray(x)
    nbr = np.asarray(nbr).astype(np.int32)
    subl = np.asarray(subl).astype(np.int32)
    red = np.asarray(red).astype(np.int32)
    W = [np.asarray(w, dtype=np.float32) for w in (W0, W1, W2, W3)]
    bvec = [np.asarray(b, dtype=np.float32) for b in (b0, b1, b2, b3)]
    Wt = np.asarray(Wt, dtype=np.float32)
    bt = np.asarray(bt, dtype=np.float32)

    # ---- host: fold gather/sublattice/readout into dense matrices ----
    M0 = _build_dense(W[0], subl, nbr)               # [1152, 36]
    Ms = [_build_dense(W[l], subl, nbr) for l in (1, 2, 3)]
    Bs = [b[subl].T.reshape(-1).astype(np.float32) for b in bvec]  # [1152]
    Rm = np.zeros((2, C, N), dtype=np.float32)
    for t in range(3):
        for i in range(red.shape[1]):
            Rm[:, :, red[t, i]] += Wt[:, np.arange(C) * 3 + t] / red.shape[1]
    Rm = Rm.reshape(2, F)

    # spins, transposed to [36, B] fp16
    bits = (x[:, None] >> np.arange(N, dtype=x.dtype)) & 1
    sT_full = np.ascontiguousarray(
        (2 * bits - 1).astype(NP16).T)         # [36, 16384]

    w0_h = np.ascontiguousarray(M0.T.astype(NP16))        # [36, 1152]
    # [128p, kc*F + c] with w_h[p, kc*F + c] = MT[kc*128+p, c]
    w_h = [np.ascontiguousarray(
        M.T.astype(NP16).reshape(KC, 128, F)
        .transpose(1, 0, 2).reshape(128, KC * F))
        for M in Ms]
    rt_h = np.zeros((128, KC * 2), dtype=NP16)            # RT tiled
    RT = Rm.T.astype(NP16)                                # [1152, 2]
    for kc in range(KC):
        rt_h[:, kc * 2:(kc + 1) * 2] = RT[kc * 128:(kc + 1) * 128, :]
    bias_h = np.zeros((128, 4 * KC), dtype=np.float32)
    for l in range(4):
        for mo in range(KC):
            bias_h[:, l * KC + mo] = Bs[l][mo * 128:(mo + 1) * 128]
    bt_h = bt.reshape(2, 1).astype(np.float32)

    nc = _build_program()
    shared = {"w0": w0_h, "w1": w_h[0], "w2": w_h[1], "w3": w_h[2],
              "rt": rt_h, "bias": bias_h, "bt": bt_h}
    in_maps = []
    for c in range(NCORES):
        m = dict(shared)
        m["sT"] = np.ascontiguousarray(sT_full[:, c * BC:(c + 1) * BC])
        in_maps.append(m)

    global _LAST_IN_MAPS
    _LAST_IN_MAPS = in_maps
    res = run_bass_kernel_spmd(nc, in_maps, list(range(NCORES)))

    out = np.empty((B, 2), dtype=np.float32)
    for c in range(NCORES):
        out[c * BC:(c + 1) * BC] = res.results[c]["outT"].T
    return out


# revision 15
# speedup vs baseline: 1.0215x; 1.0159x over previous
"""Trainium2 Bass kernel for nn_KagomeSignNetwork.

Strategy
--------
The reference network is, per sample b:
    h0[b]   = spins(x[b])                       in {-1,+1}^36
    h_{l+1} = relu(Conv_l(h_l))                 4 graph-conv layers, C=32
    out     = readout(h4)                       sublattice-mean + linear -> 2

Each graph conv  out[b,o,n] = sum_{c,k} h[b,c,nbr[n,k]] * W[subl[n],o,c,k] + b
is LINEAR in the flattened feature vector f = (c*36 + site).  We therefore
fold the neighbor gather + per-sublattice weight selection into one dense
matrix M_l [1152, 1152] on the host (and the bit-unpack / readout likewise),
turning the whole network into a dense MLP:

    h1 = relu(M0 @ s + B0)        M0: [1152, 36]
    h2 = relu(M1 @ h1 + B1)       M1..M3: [1152, 1152]
    h3 = relu(M2 @ h2 + B2)
    h4 = relu(M3 @ h3 + B3)
    out = R @ h4 + bt             R: [2, 1152]

This maps perfectly onto the 128x128 PE array (no gathers on device).
Matmuls run in fp16 (weights+activations) with fp32 PSUM accumulation:
measured end-to-end rel-l2 error vs the fp32 reference is ~2.6e-4.

Sharding: pure data parallelism — batch 16384 is split 8 x 2048; weights
replicated.  Activations live entirely in SBUF as [features, batch] tiles
(feature chunk on the 128-partition dim), so HBM traffic is just weights
(~8 MB/core) + spins in + logits out.
"""

import numpy as np

import concourse.bass as bass
import concourse.mybir as mybir
from concourse import bacc
from concourse.bass_utils import run_bass_kernel_spmd
from concourse.tile import TileContext

B, N, C = 16384, 36, 32
F = C * N            # 1152 features per layer
NCORES = 8
BC = B // NCORES     # 2048 samples per core
KC = F // 128        # 9 feature chunks of 128
NB = BC // 512       # 4 batch chunks of 512
import os
if os.environ.get("KERNEL_MM_DTYPE") == "bf16":
    import ml_dtypes
    FP16 = mybir.dt.bfloat16
    NP16 = np.dtype(ml_dtypes.bfloat16)
else:
    FP16 = mybir.dt.float16
    NP16 = np.dtype("float16")
FP32 = mybir.dt.float32

_COMPILED = {}


def _build_program():
    """Build + compile the SPMD Bass program (cached per process)."""
    if "nc" in _COMPILED:
        return _COMPILED["nc"]

    nc = bacc.Bacc("TRN2", target_bir_lowering=False, debug=False,
                   num_devices=NCORES)

    sT_d = nc.declare_dram_parameter("sT", [N, BC], FP16, isOutput=False)
    w0_d = nc.declare_dram_parameter("w0", [N, F], FP16, isOutput=False)
    w_d = [nc.declare_dram_parameter(f"w{l}", [128, KC * F], FP16,
                                     isOutput=False) for l in (1, 2, 3)]
    rt_d = nc.declare_dram_parameter("rt", [128, KC * 2], FP16, isOutput=False)
    bias_d = nc.declare_dram_parameter("bias", [128, 4 * KC], FP32,
                                       isOutput=False)
    bt_d = nc.declare_dram_parameter("bt", [128, 1], FP32, isOutput=False)
    out_d = nc.declare_dram_parameter("outT", [2, BC], FP32, isOutput=True)

    with TileContext(nc) as tc:
        with (
            tc.tile_pool(name="const", bufs=1) as cpool,
            tc.tile_pool(name="hbuf", bufs=2) as hpool,
            tc.tile_pool(name="psum", bufs=6, space="PSUM") as pspool,
            tc.tile_pool(name="psr", bufs=2, space="PSUM") as psrpool,
        ):
            # ---- resident constants / weights ----
            sT = cpool.tile([N, BC], FP16, tag="sT")
            nc.sync.dma_start(out=sT[:], in_=sT_d[:])
            w0 = cpool.tile([N, F], FP16, tag="w0")
            nc.sync.dma_start(out=w0[:], in_=w0_d[:])
            bias = cpool.tile([128, 4 * KC], FP32, tag="bias")
            nc.sync.dma_start(out=bias[:], in_=bias_d[:])
            bt = cpool.tile([128, 1], FP32, tag="bt")
            nc.sync.dma_start(out=bt[:], in_=bt_d[:])
            rt = cpool.tile([128, KC * 2], FP16, tag="rt")
            nc.sync.dma_start(out=rt[:], in_=rt_d[:])
            ws = []
            for l in range(3):
                # SBUF col mo*F + kc*128 + j  <-  MT[kc*128+p, mo*128+j]
                # (host pre-arranges w_d as [p, mo, kc, j]); one DMA per mo
                # chunk so layer l+1 can start as soon as its first output
                # chunk's weights land.
                w = cpool.tile([128, KC * F], FP16, tag=f"wl{l}")
                for mo in range(KC):
                    nc.sync.dma_start(out=w[:, mo * F:(mo + 1) * F],
                                      in_=w_d[l][:, mo * F:(mo + 1) * F])
                ws.append(w)

            relu = mybir.ActivationFunctionType.Relu
            ident = mybir.ActivationFunctionType.Identity

            # ---- PE warmup: ~3.5us of dummy matmuls during the DMA
            # window, so the HAM clock-gate is at 8/8 (2.4 GHz) by the
            # time real work starts ----
            wu = cpool.tile([128, 64], FP16, tag="wu")
            nc.vector.memset(wu[:], 0.0)
            pswu = psrpool.tile([64, 64], FP32, tag="psr")
            for _ in range(64):
                nc.tensor.matmul(pswu[:], lhsT=wu[:, :64], rhs=wu[:],
                                 start=True, stop=True)

            # ---- layer 0: K=36, s -> h1 ----
            h_prev = hpool.tile([128, KC * BC], FP16, tag="h")
            for mo in range(KC):
                for nb in range(NB):
                    ps = pspool.tile([128, 512], FP32, tag="ps")
                    nc.tensor.matmul(
                        ps[:],
                        lhsT=w0[:, mo * 128:(mo + 1) * 128],
                        rhs=sT[:, nb * 512:(nb + 1) * 512],
                        start=True, stop=True,
                    )
                    nc.scalar.activation(
                        h_prev[:, mo * BC + nb * 512: mo * BC + (nb + 1) * 512],
                        ps[:], relu, bias=bias[:, mo:mo + 1],
                    )

            # ---- layers 1..3: dense 1152 -> 1152 ----
            for l in range(3):
                w = ws[l]
                h_next = hpool.tile([128, KC * BC], FP16, tag="h")
                for mo in range(KC):
                    for nb in range(NB):
                        ps = pspool.tile([128, 512], FP32, tag="ps")
                        for kc in range(KC):
                            nc.tensor.matmul(
                                ps[:],
                                lhsT=w[:, mo * F + kc * 128:
                                       mo * F + (kc + 1) * 128],
                                rhs=h_prev[:, kc * BC + nb * 512:
                                           kc * BC + (nb + 1) * 512],
                                start=(kc == 0), stop=(kc == KC - 1),
                            )
                        nc.scalar.activation(
                            h_next[:, mo * BC + nb * 512:
                                   mo * BC + (nb + 1) * 512],
                            ps[:], relu,
                            bias=bias[:, (l + 1) * KC + mo:(l + 1) * KC + mo + 1],
                        )
                h_prev = h_next

            # ---- readout: [2, 1152] @ h4 + bt ----
            # M=2 wastes 126 of the PE's 128 columns, so run the 4 batch
            # chunks concurrently in 4 distinct 32-column groups via
            # tile_position; each lands at PSUM partitions [32j, 32j+2).
            outT = cpool.tile([128, 512], FP32, tag="outT")
            ps = psrpool.tile([128, 512], FP32, tag="psr")
            for kc in range(KC):
                for j in range(NB):
                    nc.tensor.matmul(
                        ps[32 * j:32 * j + 2, :],
                        lhsT=rt[:, kc * 2:(kc + 1) * 2],
                        rhs=h_prev[:, kc * BC + j * 512:
                                   kc * BC + (j + 1) * 512],
                        start=(kc == 0), stop=(kc == KC - 1),
                        tile_position=(0, 32 * j),
                    )
            for j in range(NB):
                nc.scalar.activation(
                    outT[32 * j:32 * j + 2, :], ps[32 * j:32 * j + 2, :],
                    ident, bias=bt[32 * j:32 * j + 2, :],
                )
                nc.sync.dma_start(out=out_d[:, j * 512:(j + 1) * 512],
                                  in_=outT[32 * j:32 * j + 2, :])

    nc.compile()
    _COMPILED["nc"] = nc
    return nc


def _build_dense(W, subl, nbr):
    """M[(o*N+n), (c*N+m)] = sum_k W[subl[n],o,c,k] * [nbr[n,k]==m]."""
    Cout, Cin = W.shape[1], W.shape[2]
    Ws = W[subl]                                     # [N, Cout, Cin, 15]
    M = np.zeros((Cout, N, Cin, N), dtype=np.float32)
    n_idx = np.repeat(np.arange(N), nbr.shape[1])
    m_idx = nbr.reshape(-1).astype(np.int64)
    vals = Ws.transpose(0, 3, 1, 2).reshape(N * nbr.shape[1], Cout, Cin)
    Mp = M.transpose(1, 3, 0, 2)                     # [N_out, N_in, Cout, Cin]
    np.add.at(Mp, (n_idx, m_idx), vals.astype(np.float32))
    return M.reshape(Cout * N, Cin * N)


def kernel(x, W0, b0, W1, b1, W2, b2, W3, b3, Wt, bt, nbr, subl, red):
    x = np.asarray(x)
    nbr = np.asarray(nbr).astype(np.int32)
    subl = np.asarray(subl).astype(np.int32)
    red = np.asarray(red).astype(np.int32)
    W = [np.asarray(w, dtype=np.float32) for w in (W0, W1, W2, W3)]
    bvec = [np.asarray(b, dtype=np.float32) for b in (b0, b1, b2, b3)]
    Wt = np.asarray(Wt, dtype=np.float32)
    bt = np.asarray(bt, dtype=np.float32)

    # ---- host: fold gather/sublattice/readout into dense matrices ----
    M0 = _build_dense(W[0], subl, nbr)               # [1152, 36]
    Ms = [_build_dense(W[l], subl, nbr) for l in (1, 2, 3)]
    Bs = [b[subl].T.reshape(-1).astype(np.float32) for b in bvec]  # [1152]
    Rm = np.zeros((2, C, N), dtype=np.float32)
    for t in range(3):
        for i in range(red.shape[1]):
            Rm[:, :, red[t, i]] += Wt[:, np.arange(C) * 3 + t] / red.shape[1]
    Rm = Rm.reshape(2, F)

    # spins, transposed to [36, B] fp16
    bits = (x[:, None] >> np.arange(N, dtype=x.dtype)) & 1
    sT_full = np.ascontiguousarray(
        (2 * bits - 1).astype(NP16).T)         # [36, 16384]

    w0_h = np.ascontiguousarray(M0.T.astype(NP16))        # [36, 1152]
    # [128p, mo*F + kc*128 + j] with value MT[kc*128+p, mo*128+j]
    w_h = [np.ascontiguousarray(
        M.T.astype(NP16).reshape(KC, 128, KC, 128)       # [kc, p, mo, j]
        .transpose(1, 2, 0, 3).reshape(128, KC * F))
        for M in Ms]
    rt_h = np.zeros((128, KC * 2), dtype=NP16)            # RT tiled
    RT = Rm.T.astype(NP16)                                # [1152, 2]
    for kc in range(KC):
        rt_h[:, kc * 2:(kc + 1) * 2] = RT[kc * 128:(kc + 1) * 128, :]
    bias_h = np.zeros((128, 4 * KC), dtype=np.float32)
    for l in range(4):
        for mo in range(KC):
            bias_h[:, l * KC + mo] = Bs[l][mo * 128:(mo + 1) * 128]
    # bt replicated at partitions {32j, 32j+1} for the col-tiled readout
    bt_h = np.zeros((128, 1), dtype=np.float32)
    for j in range(NB):
        bt_h[32 * j:32 * j + 2, 0] = bt


    nc = _build_program()
    shared = {"w0": w0_h, "w1": w_h[0], "w2": w_h[1], "w3": w_h[2],
              "rt": rt_h, "bias": bias_h, "bt": bt_h}
    in_maps = []
    for c in range(NCORES):
        m = dict(shared)
        m["sT"] = np.ascontiguousarray(sT_full[:, c * BC:(c + 1) * BC])
        in_maps.append(m)

    global _LAST_IN_MAPS
    _LAST_IN_MAPS = in_maps
    res = run_bass_kernel_spmd(nc, in_maps, list(range(NCORES)))

    out = np.empty((B, 2), dtype=np.float32)
    for c in range(NCORES):
        out[c * BC:(c + 1) * BC] = res.results[c]["outT"].T
    return out
